# revision 26
# baseline (speedup 1.0000x reference)
"""Causal multihead self-attention with RoPE on 8 TRN2 NeuronCores.

Problem: B=2, S=2048, D=1024, H=16 heads, d_k=64, causal, RoPE theta=10000.

Sharding (Megatron-style, per hint): core c = 4*b + g handles batch b and the
4 heads [4g, 4g+4): Wq/Wk/Wv column-parallel (sliced rows of W since torch
computes x @ W.T), Wo row-parallel; each core emits a partial [S, D] output
(fp16) and the host sums the 4 partials per batch in fp32.

Device kernel (per core); matmul operands in bf16 (fp32 PSUM accumulation,
fast weight loads), everything else fp32.  Three work streams:
  A) qT/kT = W' @ x.T in a d-on-partition layout; RoPE via 6 full-width DVE
     ops per (tensor, qtile) against host-built cos/sin tables; an 8-issue
     SBUF->SBUF DMA repack rearranges rows per-head-contiguous; v packed on
     ACT as per-head [v | ones] ([ones | v] for odd heads; the ones columns
     produce softmax sums for free in the attention*V matmul).
  B) Per (qtile, pair): scoresT [128 kpos, 2, 512 q] via PE row-group
     packing (2 heads concurrent), exp on ACT (|scores|<=~40, no max
     subtraction), DVE causal-mask multiply on diagonal tiles, AV
     accumulation with a 4-chunk software skew.  At pair end the
     remaining AV chunks drain per-head with each head's normalization
     (ACT ln/exp + one DVE mul into yT) emitted immediately after its
     drain, so head 0's norm hides under head 1's matmuls and the next
     pair's first AV (WAR on the 2-buf av pool) unblocks ~2us earlier.
  C) partial = yT.T @ Wo via PE, fp16 SBUF staging, one DMA per 128-row
     block alternating sync/gpsimd queues.

The defining constraint (from trace analysis): stream A is PE+DVE-heavy
with ACT idle, stream B is ACT-bound (exp ~71us) with PE ~50%.  Run
serially they floor at ~165us; this kernel INTERLEAVES them -- B(t) chunk
emissions carry A(t+1) and C(t-1) subunits distributed between them -- so
the ACT-bound stretch hides A's PE/DVE work.  Head-of-line stalls (the
failure mode of naive weaving, 212-218us in a previous attempt) are
avoided by (a) every interleaved instruction's data deps being satisfied
at least one full tile earlier, (b) disjoint PSUM budgets: scores 2x2
banks, AV 2 banks, A-projections/C-outputs share a 2-bank pool.

Other measured findings baked in:
  - Every dma_start costs ~600ns of sequencer time on the issuing engine's
    queue (the old ~95 sync-queue issues serialized ~60us); transfers fan
    out across all 16 DMA engines regardless, so issues are batched --
    except the startup-critical xt0/wq which are split per-chunk so the
    first projection matmuls start as soon as chunk 0 lands.
  - DVE tensor_tensor is ~0.67ns/elem regardless of dtype/space (no 2x/4x
    modes materialize on HW); vector.reciprocal is ~6.5 cycles/elem (far
    slower than the ACT ln/exp pair).
  - The BIR verifier requires SBUF DMA APs to keep the partition dim
    outermost; partition-block-strided out APs wedge the device
    (NRT_EXEC_UNIT_UNRECOVERABLE).
  - PE p-state drops to ~1.2GHz after idle gaps (~3us continuous execution
    to ramp); a dense interleaved stream is the fix, plus a short warmup
    during the initial DMA wait.
"""
import sys

sys.path.insert(0, "/opt/trn_rl_repo")

import numpy as np

import concourse.bacc as bacc
import concourse.hw_specs as hw_specs
import concourse.tile as tile
from concourse import mybir
from concourse.bass_utils import run_bass_kernel_spmd

# Keep Exp and Ln in one activation-table set: hide them from every other set
# so bacc's table-load pass picks natural_log_exp_and_others for both instead
# of thrashing between exp_and_others and natural_log (~2.7us per reload).
_orig_act_tables = hw_specs.get_activation_tables


def _patched_act_tables(arch):
    _E = mybir.ActivationFunctionType.Exp
    _L = mybir.ActivationFunctionType.Ln
    out = {}
    for name, fns in _orig_act_tables(arch).items():
        if name != "natural_log_exp_and_others":
            fns = fns - {_E, _L}
        out[name] = fns
    return out


bacc.get_activation_tables = _patched_act_tables

F32 = mybir.dt.float32
F32R = mybir.dt.float32r
F16 = mybir.dt.float16
BF16 = mybir.dt.bfloat16
USE_BF16 = True
MM_DT = BF16 if USE_BF16 else F32R
EXP = mybir.ActivationFunctionType.Exp
LN = mybir.ActivationFunctionType.Ln
MUL = mybir.AluOpType.mult
ADD = mybir.AluOpType.add
SUB = mybir.AluOpType.subtract

B, S, D = 2, 2048, 1024
H, DK = 16, 64          # global heads, head dim
HL = 4                  # heads per core
GD = HL * DK            # local width 256
T = S // 512            # 4 q-tiles of 512
C = S // 128            # 16 kpos chunks of 128
DCH = D // 128          # 8 contraction chunks
THETA = 10000.0

_cache = {}


def _build_kernel():
    nc = bacc.Bacc("TRN2", target_bir_lowering=False, debug=False, num_devices=8)

    xT = nc.declare_dram_parameter("xT", [D, S], MM_DT, isOutput=False)
    wq = nc.declare_dram_parameter("wq", [D, GD], MM_DT, isOutput=False)
    wk = nc.declare_dram_parameter("wk", [D, GD], MM_DT, isOutput=False)
    wv = nc.declare_dram_parameter("wv", [D, GD], MM_DT, isOutput=False)
    wo = nc.declare_dram_parameter("wo", [GD, D], MM_DT, isOutput=False)
    ccd = nc.declare_dram_parameter("cc", [128, S], F32, isOutput=False)
    ssd = nc.declare_dram_parameter("ss", [128, S], F32, isOutput=False)
    mskd = nc.declare_dram_parameter("msk", [128, 4, 2, 512], MM_DT,
                                     isOutput=False)
    out = nc.declare_dram_parameter("out", [S, D], F16, isOutput=True)

    with tile.TileContext(nc) as tc:
        with (
            tc.tile_pool(name="consts", bufs=1) as consts,
            tc.tile_pool(name="persist", bufs=1) as persist,
            tc.tile_pool(name="xtp", bufs=4) as xtp,
            tc.tile_pool(name="rtmp", bufs=8) as rtmp,
            tc.tile_pool(name="rop", bufs=3) as rop,
            tc.tile_pool(name="ep", bufs=8) as ep,
            tc.tile_pool(name="rp", bufs=10) as rp,
            tc.tile_pool(name="osb", bufs=3) as osb,
            # PSUM: scores 2x[128,2,512] (4 banks) + AV 2x[128,512]
            # (2 banks) + a 2-bank pool shared by A-projections, C-outputs
            # and warmup = exactly 8 banks
            tc.tile_pool(name="ap", bufs=2, space="PSUM") as ap,
            tc.tile_pool(name="scp", bufs=2, space="PSUM") as scp,
            tc.tile_pool(name="avp", bufs=2, space="PSUM") as avp,
        ):
            # ---- constants; xt0/wq split per-chunk so the first
            # projection matmuls start as soon as chunk 0 lands ----
            wq_t = consts.tile([128, DCH, GD], MM_DT, tag="wq")
            wk_t = consts.tile([128, DCH, GD], MM_DT, tag="wk")
            wv_t = consts.tile([128, DCH, GD], MM_DT, tag="wv")
            wo_t = consts.tile([128, 2, D], MM_DT, tag="wo")
            cc_t = consts.tile([128, S], F32, tag="cc")
            ss_t = consts.tile([128, S], F32, tag="ss")

            xts = {}
            for t in range(T):
                xts[t] = xtp.tile([128, DCH, 512], MM_DT, tag="xt",
                                  name=f"xt{t}")
            xT_v = xT.rearrange("(c p) s -> p c s", p=128)
            wq_v = wq.rearrange("(c p) g -> p c g", p=128)
            for dd in range(2):
                nc.sync.dma_start(out=xts[0][:, dd, :], in_=xT_v[:, dd, 0:512])
                nc.sync.dma_start(out=wq_t[:, dd, :], in_=wq_v[:, dd, :])
            nc.sync.dma_start(out=xts[0][:, 2:5, :], in_=xT_v[:, 2:5, 0:512])
            nc.sync.dma_start(out=wq_t[:, 2:, :], in_=wq_v[:, 2:, :])
            nc.gpsimd.dma_start(out=xts[0][:, 5:, :], in_=xT_v[:, 5:, 0:512])
            nc.gpsimd.dma_start(out=cc_t[:], in_=ccd[:])
            nc.gpsimd.dma_start(out=ss_t[:], in_=ssd[:])
            nc.sync.dma_start(
                out=wk_t[:], in_=wk.rearrange("(c p) g -> p c g", p=128))
            nc.sync.dma_start(
                out=wv_t[:], in_=wv.rearrange("(c p) g -> p c g", p=128))
            msk_t = consts.tile([128, 4, 2, 512], MM_DT, tag="msk")
            nc.sync.dma_start(out=msk_t[:], in_=mskd[:])
            # prefetch the remaining x tiles (transfers overlap compute)
            for t in range(1, T):
                nc.sync.dma_start(out=xts[t][:],
                                  in_=xT_v[:, :, 512 * t:512 * (t + 1)])

            # PE warmup during the initial DMA wait (the PE clock needs
            # ~3us of continuous execution to ramp to 2.4GHz)
            warm = consts.tile([128, 128], MM_DT, tag="warm")
            nc.vector.memset(warm[:], 0.03125)
            wps = ap.tile([128, 512], F32, tag="ap", name="warmps")
            NWARM = 28
            for i in range(NWARM):
                nc.tensor.matmul(wps[:, 0:128], lhsT=warm[:], rhs=warm[:],
                                 start=(i == 0), stop=(i == NWARM - 1))

            kw_n = [0]

            def keepwarm(n=2):
                kw_n[0] += 1
                wt = ap.tile([128, 512], F32, tag="ap",
                             name=f"kw{kw_n[0]}")
                for i in range(n):
                    nc.tensor.matmul(wt[:, 0:128], lhsT=warm[:],
                                     rhs=warm[:], start=True, stop=True)

            ones_f = consts.tile([128, 2, DK], F32, tag="onesf")
            nc.vector.memset(ones_f[:], 1.0)
            ones = consts.tile([128, 2, DK], MM_DT, tag="ones")
            nc.vector.tensor_copy(ones[:], ones_f[:])

            # persistent activations
            qT = persist.tile([128, 2, S], MM_DT, tag="qT")
            kT = persist.tile([128, 2, S], MM_DT, tag="kT")
            yT = persist.tile([128, 2, S], MM_DT, tag="yT")
            v_ext = persist.tile([128, C, HL, 2 * DK], MM_DT, tag="vext")

            # ones halves of v_ext: even heads [64:128], odd heads [0:64]
            for c in range(C):
                for par, sl in ((0, slice(DK, 2 * DK)), (1, slice(0, DK))):
                    nc.vector.tensor_copy(v_ext[:, c, par::2, sl], ones[:])

            # ---- stream A subunit emitters (projections + RoPE + v) ----
            def a_subunits(t):
                """10 closures: for q and k: proj-oc0+mul2, proj-oc1+mul3,
                rope-final+repack; then 4 v units."""
                qs = slice(512 * t, 512 * (t + 1))
                xt = xts[t]
                subs = []
                for wi_, (w_t, dst) in enumerate(((wq_t, qT), (wk_t, kT))):
                    st = {}
                    un = f"{t}_{wi_}"

                    def proj0(w_t=w_t, st=st, t=t, un=un):
                        ccs, sss = cc_t[:, qs], ss_t[:, qs]
                        st["t1"] = rtmp.tile([128, 512], F32, tag="rt",
                                             name=f"t1_{un}")
                        st["t2"] = rtmp.tile([128, 512], F32, tag="rt",
                                             name=f"t2_{un}")
                        st["t3"] = rtmp.tile([128, 512], F32, tag="rt",
                                             name=f"t3_{un}")
                        st["t4"] = rtmp.tile([128, 512], F32, tag="rt",
                                             name=f"t4_{un}")
                        st["ro"] = rop.tile([128, 2, 512], MM_DT, tag="ro",
                                            name=f"ro_{un}")
                        ps = ap.tile([128, 512], F32, tag="ap",
                                     name=f"ps0_{un}")
                        for d in range(DCH):
                            nc.tensor.matmul(
                                ps[:], lhsT=w_t[:, d, 0:128],
                                rhs=xt[:, d, :],
                                start=(d == 0), stop=(d == DCH - 1))
                        nc.vector.tensor_tensor(st["t1"][:], ps[:], ccs,
                                                op=MUL)
                        nc.vector.tensor_tensor(st["t3"][:], ps[:], sss,
                                                op=MUL)

                    def proj1(w_t=w_t, st=st, t=t, un=un):
                        ccs, sss = cc_t[:, qs], ss_t[:, qs]
                        ps = ap.tile([128, 512], F32, tag="ap",
                                     name=f"ps1_{un}")
                        for d in range(DCH):
                            nc.tensor.matmul(
                                ps[:], lhsT=w_t[:, d, 128:256],
                                rhs=xt[:, d, :],
                                start=(d == 0), stop=(d == DCH - 1))
                        nc.vector.tensor_tensor(st["t2"][:], ps[:], sss,
                                                op=MUL)
                        nc.vector.tensor_tensor(st["ro"][:, 0, :],
                                                st["t1"][:], st["t2"][:],
                                                op=SUB)
                        nc.vector.tensor_tensor(st["t4"][:], ps[:], ccs,
                                                op=MUL)

                    def ropef(dst=dst, st=st, t=t):
                        nc.vector.tensor_tensor(st["ro"][:, 1, :],
                                                st["t3"][:], st["t4"][:],
                                                op=ADD)
                        # repack to per-head-contiguous rows: dst rows
                        # 64*hp + 32*half + j, chunk oc, from ro rows
                        # 32*(2*oc+hp) + j, chunk half
                        dma_eng = nc.gpsimd if dst is qT else nc.sync
                        ro = st["ro"]
                        for half in range(2):
                            for oc in range(2):
                                for hp in range(2):
                                    sp = 32 * (2 * oc + hp)
                                    dp = 64 * hp + 32 * half
                                    dma_eng.dma_start(
                                        out=dst[dp:dp + 32, oc, qs],
                                        in_=ro[sp:sp + 32, half, :],
                                    )

                    subs += [proj0, proj1, ropef]

                for s4 in range(4):
                    def vunit(s4=s4, t=t):
                        s = 4 * t + s4
                        psv = ap.tile([128, 512], F32, tag="ap",
                                      name=f"psv_{t}_{s4}")
                        for d in range(DCH):
                            nc.tensor.matmul(
                                psv[:, :GD],
                                lhsT=xt[:, d, 128 * s4:128 * (s4 + 1)],
                                rhs=wv_t[:, d, :],
                                start=(d == 0), stop=(d == DCH - 1))
                        pv = psv[:, :GD].rearrange("p (h e) -> p h e", e=DK)
                        # v packing on ACT (slack there; DVE carries RoPE)
                        for par, sl in ((0, slice(0, DK)),
                                        (1, slice(DK, 2 * DK))):
                            nc.scalar.copy(
                                v_ext[:, s, par::2, sl], pv[:, par::2, :])
                    subs.append(vunit)
                return subs

            # ---- stream C subunit emitters (output projection) ----
            def c_subunits(t, tail=False):
                """8 closures, one per (row-block, half): po matmuls + fp16
                staging; the DMA (one per row-block) fires after half 1."""
                subs = []
                for s_ in range(4 * t, 4 * t + 4):
                    ob = osb.tile([128, 2, 512], F16, tag="ob",
                                  name=f"ob_{s_}")

                    def unit(s_=s_, ob=ob, n=0):
                        po = ap.tile([128, 512], F32, tag="ap",
                                     name=f"po_{s_}_{n}")
                        for ldc in range(2):
                            nc.tensor.matmul(
                                po[:],
                                lhsT=yT[:, ldc, 128 * s_:128 * (s_ + 1)],
                                rhs=wo_t[:, ldc, 512 * n:512 * (n + 1)],
                                start=(ldc == 0), stop=(ldc == 1))
                        if tail and n == 1:
                            # ACT is idle in the tail; splitting the copies
                            # shortens the serial epilogue chain
                            nc.scalar.copy(ob[:, n, :], po[:])
                        else:
                            nc.vector.tensor_copy(ob[:, n, :], po[:])
                        if n == 1:
                            nc.sync.dma_start(
                                out=out[128 * s_:128 * (s_ + 1), :],
                                in_=ob[:])

                    subs.append(lambda unit=unit: unit(n=0))
                    subs.append(lambda unit=unit: unit(n=1))
                return subs

            # ---- stream B emitter for one (t, pair) with interleaved
            # work items pulled between chunk emissions ----
            def b_pair(t, pair, work, wi):
                qs = slice(512 * t, 512 * (t + 1))
                heads = (2 * pair, 2 * pair + 1)
                av_ps = {h: avp.tile([128, 512], F32, tag="av",
                                     name=f"av_{t}_{h}")
                         for h in heads}
                nck = 4 * t + 4
                nwork = len(work)
                ntot = 8 * t + 8  # chunks in this tile (both pairs)
                pend = []
                for c in range(nck):
                    ks = slice(128 * c, 128 * (c + 1))
                    j = c - 4 * t
                    f0 = 128 * j if (0 < j < 4 and c > 0) else 0
                    sc = scp.tile([128, 2, 512], F32, tag="sc",
                                  name=f"sc_{t}_{pair}_{c}")
                    for hp in range(2):
                        rows = slice(64 * hp, 64 * hp + 64)
                        nc.tensor.matmul(
                            sc[:, hp, f0:],
                            lhsT=kT[rows, pair, ks],
                            rhs=qT[rows, pair, 512 * t + f0:512 * (t + 1)],
                            start=True, stop=True,
                            tile_position=(64 * hp, 0),
                        )
                    e = ep.tile([128, 2, 512], MM_DT, tag="e",
                                name=f"e_{t}_{pair}_{c}")
                    nc.scalar.activation(e[:, :, f0:], sc[:, :, f0:], EXP)
                    if c >= 4 * t:
                        nc.vector.tensor_tensor(
                            e[:, :, f0:], e[:, :, f0:],
                            msk_t[:, j, :, f0:], op=MUL)
                    pend.append((c, e, f0))
                    if len(pend) > 4:
                        pc, pe_, pf0 = pend.pop(0)
                        for hp, h in enumerate(heads):
                            nc.tensor.matmul(
                                av_ps[h][:, pf0:],
                                lhsT=v_ext[:, pc, h, :],
                                rhs=pe_[:, hp, pf0:],
                                start=(pc == 0), stop=False)
                    # pull interleaved A/C work: distribute the nwork
                    # items evenly across this tile's ntot chunks
                    done = pair * nck + c + 1
                    target = min(nwork, (done * nwork) // (ntot + 2))
                    while wi[0] < target:
                        work[wi[0]]()
                        wi[0] += 1
                last_pc = pend[-1][0]
                for hp, h in enumerate(heads):
                    for pc, pe_, pf0 in pend:
                        nc.tensor.matmul(
                            av_ps[h][:, pf0:],
                            lhsT=v_ext[:, pc, h, :],
                            rhs=pe_[:, hp, pf0:],
                            start=(pc == 0), stop=(pc == last_pc))
                    if h % 2 == 0:
                        srows, orows = slice(64, 128), slice(0, 64)
                    else:
                        srows, orows = slice(0, 64), slice(64, 128)
                    r1 = rp.tile([128, 512], F32, tag="rr",
                                 name=f"r1_{t}_{h}")
                    r2 = rp.tile([128, 512], F32, tag="rr",
                                 name=f"r2_{t}_{h}")
                    nc.scalar.activation(r1[srows], av_ps[h][srows], LN)
                    nc.scalar.activation(r2[srows], r1[srows], EXP,
                                         scale=-1.0)
                    # psum + sbuf operands may use different base partitions
                    nc.vector.tensor_tensor(
                        yT[orows, h // 2, qs],
                        av_ps[h][orows], r2[srows], op=MUL)
                pend = []
                # post-norm work item: gives the PE queue something to chew
                # while the DVE norm muls drain (next pair's first AV
                # matmul WAR-waits on them through the 2-buf av pool)
                if wi[0] < nwork:
                    work[wi[0]]()
                    wi[0] += 1
                else:
                    # no interleavable work left (t=3): a few dummies keep
                    # the PE p-state alive through the norm chain
                    keepwarm(4)

            # ---- prologue: stream A for tile 0, undiluted ----
            for sub in a_subunits(0):
                sub()

            # ---- main: B(t) carrying A(t+1) and C(t-1) ----
            for t in range(T):
                work = []
                if t >= 1:
                    work += c_subunits(t - 1)
                if t <= 2:
                    work += a_subunits(t + 1)
                if t == 0:
                    nc.sync.dma_start(
                        out=wo_t[:],
                        in_=wo.rearrange("(c p) d -> p c d", p=128))
                wi = [0]
                for pair in range(2):
                    b_pair(t, pair, work, wi)
                while wi[0] < len(work):
                    work[wi[0]]()
                    wi[0] += 1

            # ---- tail: C(3) ----
            for i, sub in enumerate(c_subunits(3, tail=True)):
                sub()

    nc.compile()
    return nc


def _host_prep(x, token_positions, Wq, Wk, Wv, Wo):
    # d_k permutation folded into Wq/Wk.  Projection-output row n (0..255):
    # chunk oc = n//128 (all x1 lanes in chunk 0, x2 in chunk 1 for RoPE),
    # head h = (n%128)//32, freq j = n%32 -> orig row 64h + 2j + oc.
    # (The post-RoPE repack DMA then rearranges rows per-head-contiguous.)
    n = np.arange(GD)
    chunk = n // 128
    hh = (n % 128) // 32
    jj = n % 32
    perm = 64 * hh + 2 * jj + chunk

    pos = np.asarray(token_positions).astype(np.float64)
    inv_freq = THETA ** (-np.arange(0, DK, 2, dtype=np.float64) / DK)  # [32]
    ang = pos[:, None] * inv_freq[None, :]                             # [S, 32]
    cos = np.cos(ang).astype(np.float32)
    sin = np.sin(ang).astype(np.float32)
    # [128, S]: rows = freq j, replicated x4 on the host so the device
    # loads each table with a single DMA
    cc = np.ascontiguousarray(np.tile(cos.T, (4, 1)))
    ss = np.ascontiguousarray(np.tile(sin.T, (4, 1)))

    # causal mask tiles for diagonal-crossing chunks: msk[p, j, :, f] = 1.0
    # iff f >= p + 128*j (duplicated across the head-pair dim)
    pp_, ff_ = np.arange(128)[:, None], np.arange(512)[None, :]
    msk1 = np.stack([(ff_ >= pp_ + 128 * j) for j in range(4)], 0)  # [4,128,512]
    msk = np.repeat(msk1.transpose(1, 0, 2)[:, :, None, :], 2, axis=2)

    scale = 1.0 / np.sqrt(np.float32(DK))
    if USE_BF16:
        import ml_dtypes
        mmnp = ml_dtypes.bfloat16
    else:
        mmnp = np.float32
    in_maps = []
    for core in range(8):
        b, g = divmod(core, 4)
        gsl = slice(GD * g, GD * (g + 1))
        in_maps.append({
            "xT": np.ascontiguousarray(np.asarray(x[b], np.float32).T).astype(mmnp),
            "wq": np.ascontiguousarray(
                (np.asarray(Wq[gsl], np.float32) * scale)[perm].T.astype(mmnp)),
            "wk": np.ascontiguousarray(np.asarray(Wk[gsl], np.float32)[perm].T.astype(mmnp)),
            "wv": np.ascontiguousarray(np.asarray(Wv[gsl], np.float32).T.astype(mmnp)),
            "wo": np.ascontiguousarray(np.asarray(Wo[:, gsl], np.float32).T.astype(mmnp)),
            "cc": cc,
            "ss": ss,
            "msk": np.ascontiguousarray(msk).astype(mmnp),
        })
    return in_maps


def kernel(x, token_positions, Wq, Wk, Wv, Wo, _trace=False, _result=[None],
           _tmpdir=None):
    if "nc" not in _cache:
        _cache["nc"] = _build_kernel()
    nc = _cache["nc"]
    in_maps = _host_prep(x, token_positions, Wq, Wk, Wv, Wo)
    res = None
    for attempt in range(3):
        try:
            res = run_bass_kernel_spmd(
                nc, in_maps, core_ids=list(range(8)), trace=_trace,
                tmpdir=_tmpdir)
            break
        except Exception:
            # transient NRT_EXEC_UNIT_UNRECOVERABLE device hiccups resolve
            # on retry
            if attempt == 2:
                raise
    _result[0] = res
    outs = np.stack([r["out"] for r in res.results])  # [8, S, D] fp16
    full = outs.reshape(B, 4, S, D).sum(axis=1, dtype=np.float32)
    return full


# revision 27
# speedup vs baseline: 1.0102x; 1.0102x over previous
"""Causal multihead self-attention with RoPE on 8 TRN2 NeuronCores.

Problem: B=2, S=2048, D=1024, H=16 heads, d_k=64, causal, RoPE theta=10000.

Sharding (Megatron-style, per hint): core c = 4*b + g handles batch b and the
4 heads [4g, 4g+4): Wq/Wk/Wv column-parallel (sliced rows of W since torch
computes x @ W.T), Wo row-parallel; each core emits a partial [S, D] output
(fp16) and the host sums the 4 partials per batch in fp32.

Device kernel (per core); matmul operands in bf16 (fp32 PSUM accumulation,
fast weight loads), everything else fp32.  Three work streams:
  A) qT/kT = W' @ x.T in a d-on-partition layout; RoPE via 6 full-width DVE
     ops per (tensor, qtile) against host-built cos/sin tables; an 8-issue
     SBUF->SBUF DMA repack rearranges rows per-head-contiguous; v packed on
     ACT as per-head [v | ones] ([ones | v] for odd heads; the ones columns
     produce softmax sums for free in the attention*V matmul).
  B) Per (qtile, pair): scoresT [128 kpos, 2, 512 q] via PE row-group
     packing (2 heads concurrent), exp on ACT (|scores|<=~40, no max
     subtraction), DVE causal-mask multiply on diagonal tiles, AV
     accumulation with a 4-chunk software skew.  At pair end the
     remaining AV chunks drain per-head with each head's normalization
     (ACT ln/exp + one DVE mul into yT) emitted immediately after its
     drain, so head 0's norm hides under head 1's matmuls and the next
     pair's first AV (WAR on the 2-buf av pool) unblocks ~2us earlier.
  C) partial = yT.T @ Wo via PE, fp16 SBUF staging, one DMA per 128-row
     block alternating sync/gpsimd queues.

The defining constraint (from trace analysis): stream A is PE+DVE-heavy
with ACT idle, stream B is ACT-bound (exp ~71us) with PE ~50%.  Run
serially they floor at ~165us; this kernel INTERLEAVES them -- B(t) chunk
emissions carry A(t+1) and C(t-1) subunits distributed between them -- so
the ACT-bound stretch hides A's PE/DVE work.  Head-of-line stalls (the
failure mode of naive weaving, 212-218us in a previous attempt) are
avoided by (a) every interleaved instruction's data deps being satisfied
at least one full tile earlier, (b) disjoint PSUM budgets: scores 2x2
banks, AV 2 banks, A-projections/C-outputs share a 2-bank pool.

Other measured findings baked in:
  - Every dma_start costs ~600ns of sequencer time on the issuing engine's
    queue (the old ~95 sync-queue issues serialized ~60us); transfers fan
    out across all 16 DMA engines regardless, so issues are batched --
    except the startup-critical xt0/wq which are split per-chunk so the
    first projection matmuls start as soon as chunk 0 lands.
  - DVE tensor_tensor is ~0.67ns/elem regardless of dtype/space (no 2x/4x
    modes materialize on HW); vector.reciprocal is ~6.5 cycles/elem (far
    slower than the ACT ln/exp pair).
  - The BIR verifier requires SBUF DMA APs to keep the partition dim
    outermost; partition-block-strided out APs wedge the device
    (NRT_EXEC_UNIT_UNRECOVERABLE).
  - PE p-state drops to ~1.2GHz after idle gaps (~3us continuous execution
    to ramp); a dense interleaved stream is the fix, plus a short warmup
    during the initial DMA wait.
"""
import sys

sys.path.insert(0, "/opt/trn_rl_repo")

import numpy as np

import concourse.bacc as bacc
import concourse.hw_specs as hw_specs
import concourse.tile as tile
from concourse import mybir
from concourse.bass_utils import run_bass_kernel_spmd

# Keep Exp and Ln in one activation-table set: hide them from every other set
# so bacc's table-load pass picks natural_log_exp_and_others for both instead
# of thrashing between exp_and_others and natural_log (~2.7us per reload).
_orig_act_tables = hw_specs.get_activation_tables


def _patched_act_tables(arch):
    _E = mybir.ActivationFunctionType.Exp
    _L = mybir.ActivationFunctionType.Ln
    out = {}
    for name, fns in _orig_act_tables(arch).items():
        if name != "natural_log_exp_and_others":
            fns = fns - {_E, _L}
        out[name] = fns
    return out


bacc.get_activation_tables = _patched_act_tables

F32 = mybir.dt.float32
F32R = mybir.dt.float32r
F16 = mybir.dt.float16
BF16 = mybir.dt.bfloat16
USE_BF16 = True
MM_DT = BF16 if USE_BF16 else F32R
EXP = mybir.ActivationFunctionType.Exp
LN = mybir.ActivationFunctionType.Ln
MUL = mybir.AluOpType.mult
ADD = mybir.AluOpType.add
SUB = mybir.AluOpType.subtract

B, S, D = 2, 2048, 1024
H, DK = 16, 64          # global heads, head dim
HL = 4                  # heads per core
GD = HL * DK            # local width 256
T = S // 512            # 4 q-tiles of 512
C = S // 128            # 16 kpos chunks of 128
DCH = D // 128          # 8 contraction chunks
THETA = 10000.0

_cache = {}


def _build_kernel():
    nc = bacc.Bacc("TRN2", target_bir_lowering=False, debug=False, num_devices=8)

    xT = nc.declare_dram_parameter("xT", [D, S], MM_DT, isOutput=False)
    wq = nc.declare_dram_parameter("wq", [D, GD], MM_DT, isOutput=False)
    wk = nc.declare_dram_parameter("wk", [D, GD], MM_DT, isOutput=False)
    wv = nc.declare_dram_parameter("wv", [D, GD], MM_DT, isOutput=False)
    wo = nc.declare_dram_parameter("wo", [GD, D], MM_DT, isOutput=False)
    ccd = nc.declare_dram_parameter("cc", [128, S], F32, isOutput=False)
    ssd = nc.declare_dram_parameter("ss", [128, S], F32, isOutput=False)
    mskd = nc.declare_dram_parameter("msk", [128, 4, 2, 512], MM_DT,
                                     isOutput=False)
    out = nc.declare_dram_parameter("out", [S, D], F16, isOutput=True)

    with tile.TileContext(nc) as tc:
        with (
            tc.tile_pool(name="consts", bufs=1) as consts,
            tc.tile_pool(name="persist", bufs=1) as persist,
            tc.tile_pool(name="xtp", bufs=4) as xtp,
            tc.tile_pool(name="rtmp", bufs=8) as rtmp,
            tc.tile_pool(name="rop", bufs=3) as rop,
            tc.tile_pool(name="ep", bufs=8) as ep,
            tc.tile_pool(name="rp", bufs=10) as rp,
            tc.tile_pool(name="osb", bufs=3) as osb,
            # PSUM: scores 2x[128,2,512] (4 banks) + AV 2x[128,512]
            # (2 banks) + a 2-bank pool shared by A-projections, C-outputs
            # and warmup = exactly 8 banks
            tc.tile_pool(name="ap", bufs=2, space="PSUM") as ap,
            tc.tile_pool(name="scp", bufs=2, space="PSUM") as scp,
            tc.tile_pool(name="avp", bufs=2, space="PSUM") as avp,
        ):
            # ---- constants; xt0/wq split per-chunk so the first
            # projection matmuls start as soon as chunk 0 lands ----
            wq_t = consts.tile([128, DCH, GD], MM_DT, tag="wq")
            wk_t = consts.tile([128, DCH, GD], MM_DT, tag="wk")
            wv_t = consts.tile([128, DCH, GD], MM_DT, tag="wv")
            wo_t = consts.tile([128, 2, D], MM_DT, tag="wo")
            cc_t = consts.tile([128, S], F32, tag="cc")
            ss_t = consts.tile([128, S], F32, tag="ss")

            xts = {}
            for t in range(T):
                xts[t] = xtp.tile([128, DCH, 512], MM_DT, tag="xt",
                                  name=f"xt{t}")
            xT_v = xT.rearrange("(c p) s -> p c s", p=128)
            wq_v = wq.rearrange("(c p) g -> p c g", p=128)
            for dd in range(2):
                nc.sync.dma_start(out=xts[0][:, dd, :], in_=xT_v[:, dd, 0:512])
                nc.sync.dma_start(out=wq_t[:, dd, :], in_=wq_v[:, dd, :])
            nc.sync.dma_start(out=xts[0][:, 2:, :], in_=xT_v[:, 2:, 0:512])
            nc.sync.dma_start(out=wq_t[:, 2:, :], in_=wq_v[:, 2:, :])
            nc.gpsimd.dma_start(out=cc_t[:], in_=ccd[:])
            nc.gpsimd.dma_start(out=ss_t[:], in_=ssd[:])
            nc.sync.dma_start(
                out=wk_t[:], in_=wk.rearrange("(c p) g -> p c g", p=128))
            nc.sync.dma_start(
                out=wv_t[:], in_=wv.rearrange("(c p) g -> p c g", p=128))
            msk_t = consts.tile([128, 4, 2, 512], MM_DT, tag="msk")
            nc.sync.dma_start(out=msk_t[:], in_=mskd[:])
            # prefetch the remaining x tiles (transfers overlap compute)
            for t in range(1, T):
                nc.sync.dma_start(out=xts[t][:],
                                  in_=xT_v[:, :, 512 * t:512 * (t + 1)])

            # PE warmup during the initial DMA wait (the PE clock needs
            # ~3us of continuous execution to ramp to 2.4GHz)
            warm = consts.tile([128, 128], MM_DT, tag="warm")
            nc.vector.memset(warm[:], 0.03125)
            wps = ap.tile([128, 512], F32, tag="ap", name="warmps")
            NWARM = 28
            for i in range(NWARM):
                nc.tensor.matmul(wps[:, 0:128], lhsT=warm[:], rhs=warm[:],
                                 start=(i == 0), stop=(i == NWARM - 1))

            kw_n = [0]

            def keepwarm(n=2):
                kw_n[0] += 1
                wt = ap.tile([128, 512], F32, tag="ap",
                             name=f"kw{kw_n[0]}")
                for i in range(n):
                    nc.tensor.matmul(wt[:, 0:128], lhsT=warm[:],
                                     rhs=warm[:], start=True, stop=True)

            ones_f = consts.tile([128, 2, DK], F32, tag="onesf")
            nc.vector.memset(ones_f[:], 1.0)
            ones = consts.tile([128, 2, DK], MM_DT, tag="ones")
            nc.vector.tensor_copy(ones[:], ones_f[:])

            # persistent activations
            qT = persist.tile([128, 2, S], MM_DT, tag="qT")
            kT = persist.tile([128, 2, S], MM_DT, tag="kT")
            yT = persist.tile([128, 2, S], MM_DT, tag="yT")
            v_ext = persist.tile([128, C, HL, 2 * DK], MM_DT, tag="vext")

            # ones halves of v_ext: even heads [64:128], odd heads [0:64]
            for c in range(C):
                for par, sl in ((0, slice(DK, 2 * DK)), (1, slice(0, DK))):
                    nc.vector.tensor_copy(v_ext[:, c, par::2, sl], ones[:])

            # ---- stream A subunit emitters (projections + RoPE + v) ----
            def a_subunits(t):
                """10 closures: for q and k: proj-oc0+mul2, proj-oc1+mul3,
                rope-final+repack; then 4 v units."""
                qs = slice(512 * t, 512 * (t + 1))
                xt = xts[t]
                subs = []
                for wi_, (w_t, dst) in enumerate(((wq_t, qT), (wk_t, kT))):
                    st = {}
                    un = f"{t}_{wi_}"

                    def proj0(w_t=w_t, st=st, t=t, un=un):
                        ccs, sss = cc_t[:, qs], ss_t[:, qs]
                        st["t1"] = rtmp.tile([128, 512], F32, tag="rt",
                                             name=f"t1_{un}")
                        st["t2"] = rtmp.tile([128, 512], F32, tag="rt",
                                             name=f"t2_{un}")
                        st["t3"] = rtmp.tile([128, 512], F32, tag="rt",
                                             name=f"t3_{un}")
                        st["t4"] = rtmp.tile([128, 512], F32, tag="rt",
                                             name=f"t4_{un}")
                        st["ro"] = rop.tile([128, 2, 512], MM_DT, tag="ro",
                                            name=f"ro_{un}")
                        ps = ap.tile([128, 512], F32, tag="ap",
                                     name=f"ps0_{un}")
                        for d in range(DCH):
                            nc.tensor.matmul(
                                ps[:], lhsT=w_t[:, d, 0:128],
                                rhs=xt[:, d, :],
                                start=(d == 0), stop=(d == DCH - 1))
                        nc.vector.tensor_tensor(st["t1"][:], ps[:], ccs,
                                                op=MUL)
                        nc.vector.tensor_tensor(st["t3"][:], ps[:], sss,
                                                op=MUL)

                    def proj1(w_t=w_t, st=st, t=t, un=un):
                        ccs, sss = cc_t[:, qs], ss_t[:, qs]
                        ps = ap.tile([128, 512], F32, tag="ap",
                                     name=f"ps1_{un}")
                        for d in range(DCH):
                            nc.tensor.matmul(
                                ps[:], lhsT=w_t[:, d, 128:256],
                                rhs=xt[:, d, :],
                                start=(d == 0), stop=(d == DCH - 1))
                        nc.vector.tensor_tensor(st["t2"][:], ps[:], sss,
                                                op=MUL)
                        nc.vector.tensor_tensor(st["ro"][:, 0, :],
                                                st["t1"][:], st["t2"][:],
                                                op=SUB)
                        nc.vector.tensor_tensor(st["t4"][:], ps[:], ccs,
                                                op=MUL)

                    def ropef(dst=dst, st=st, t=t):
                        nc.vector.tensor_tensor(st["ro"][:, 1, :],
                                                st["t3"][:], st["t4"][:],
                                                op=ADD)
                        # repack to per-head-contiguous rows: dst rows
                        # 64*hp + 32*half + j, chunk oc, from ro rows
                        # 32*(2*oc+hp) + j, chunk half
                        dma_eng = nc.gpsimd if dst is qT else nc.sync
                        ro = st["ro"]
                        for half in range(2):
                            for oc in range(2):
                                for hp in range(2):
                                    sp = 32 * (2 * oc + hp)
                                    dp = 64 * hp + 32 * half
                                    dma_eng.dma_start(
                                        out=dst[dp:dp + 32, oc, qs],
                                        in_=ro[sp:sp + 32, half, :],
                                    )

                    subs += [proj0, proj1, ropef]

                for s4 in range(4):
                    def vunit(s4=s4, t=t):
                        s = 4 * t + s4
                        psv = ap.tile([128, 512], F32, tag="ap",
                                      name=f"psv_{t}_{s4}")
                        for d in range(DCH):
                            nc.tensor.matmul(
                                psv[:, :GD],
                                lhsT=xt[:, d, 128 * s4:128 * (s4 + 1)],
                                rhs=wv_t[:, d, :],
                                start=(d == 0), stop=(d == DCH - 1))
                        pv = psv[:, :GD].rearrange("p (h e) -> p h e", e=DK)
                        # v packing on ACT (slack there; DVE carries RoPE)
                        for par, sl in ((0, slice(0, DK)),
                                        (1, slice(DK, 2 * DK))):
                            nc.scalar.copy(
                                v_ext[:, s, par::2, sl], pv[:, par::2, :])
                    subs.append(vunit)
                return subs

            # ---- stream C subunit emitters (output projection) ----
            def c_subunits(t, tail=False):
                """8 closures, one per (row-block, half): po matmuls + fp16
                staging; the DMA (one per row-block) fires after half 1."""
                subs = []
                for s_ in range(4 * t, 4 * t + 4):
                    ob = osb.tile([128, 2, 512], F16, tag="ob",
                                  name=f"ob_{s_}")

                    def unit(s_=s_, ob=ob, n=0):
                        po = ap.tile([128, 512], F32, tag="ap",
                                     name=f"po_{s_}_{n}")
                        for ldc in range(2):
                            nc.tensor.matmul(
                                po[:],
                                lhsT=yT[:, ldc, 128 * s_:128 * (s_ + 1)],
                                rhs=wo_t[:, ldc, 512 * n:512 * (n + 1)],
                                start=(ldc == 0), stop=(ldc == 1))
                        if tail and n == 1:
                            # ACT is idle in the tail; splitting the copies
                            # shortens the serial epilogue chain
                            nc.scalar.copy(ob[:, n, :], po[:])
                        else:
                            nc.vector.tensor_copy(ob[:, n, :], po[:])
                        if n == 1:
                            nc.sync.dma_start(
                                out=out[128 * s_:128 * (s_ + 1), :],
                                in_=ob[:])

                    subs.append(lambda unit=unit: unit(n=0))
                    subs.append(lambda unit=unit: unit(n=1))
                return subs

            # ---- stream B emitter for one (t, pair) with interleaved
            # work items pulled between chunk emissions ----
            def b_pair(t, pair, work, wi):
                qs = slice(512 * t, 512 * (t + 1))
                heads = (2 * pair, 2 * pair + 1)
                av_ps = {h: avp.tile([128, 512], F32, tag="av",
                                     name=f"av_{t}_{h}")
                         for h in heads}
                nck = 4 * t + 4
                nwork = len(work)
                ntot = 8 * t + 8  # chunks in this tile (both pairs)
                pend = []
                for c in range(nck):
                    ks = slice(128 * c, 128 * (c + 1))
                    j = c - 4 * t
                    f0 = 128 * j if (0 < j < 4 and c > 0) else 0
                    sc = scp.tile([128, 2, 512], F32, tag="sc",
                                  name=f"sc_{t}_{pair}_{c}")
                    for hp in range(2):
                        rows = slice(64 * hp, 64 * hp + 64)
                        nc.tensor.matmul(
                            sc[:, hp, f0:],
                            lhsT=kT[rows, pair, ks],
                            rhs=qT[rows, pair, 512 * t + f0:512 * (t + 1)],
                            start=True, stop=True,
                            tile_position=(64 * hp, 0),
                        )
                    e = ep.tile([128, 2, 512], MM_DT, tag="e",
                                name=f"e_{t}_{pair}_{c}")
                    nc.scalar.activation(e[:, :, f0:], sc[:, :, f0:], EXP)
                    if c >= 4 * t:
                        nc.vector.tensor_tensor(
                            e[:, :, f0:], e[:, :, f0:],
                            msk_t[:, j, :, f0:], op=MUL)
                    pend.append((c, e, f0))
                    if len(pend) > 4:
                        pc, pe_, pf0 = pend.pop(0)
                        for hp, h in enumerate(heads):
                            nc.tensor.matmul(
                                av_ps[h][:, pf0:],
                                lhsT=v_ext[:, pc, h, :],
                                rhs=pe_[:, hp, pf0:],
                                start=(pc == 0), stop=False)
                    # pull interleaved A/C work: distribute the nwork
                    # items evenly across this tile's ntot chunks
                    done = pair * nck + c + 1
                    target = min(nwork, (done * nwork) // (ntot + 2))
                    while wi[0] < target:
                        work[wi[0]]()
                        wi[0] += 1
                last_pc = pend[-1][0]
                for hp, h in enumerate(heads):
                    for pc, pe_, pf0 in pend:
                        nc.tensor.matmul(
                            av_ps[h][:, pf0:],
                            lhsT=v_ext[:, pc, h, :],
                            rhs=pe_[:, hp, pf0:],
                            start=(pc == 0), stop=(pc == last_pc))
                    if h % 2 == 0:
                        srows, orows = slice(64, 128), slice(0, 64)
                    else:
                        srows, orows = slice(0, 64), slice(64, 128)
                    r1 = rp.tile([128, 512], F32, tag="rr",
                                 name=f"r1_{t}_{h}")
                    r2 = rp.tile([128, 512], F32, tag="rr",
                                 name=f"r2_{t}_{h}")
                    nc.scalar.activation(r1[srows], av_ps[h][srows], LN)
                    nc.scalar.activation(r2[srows], r1[srows], EXP,
                                         scale=-1.0)
                    # psum + sbuf operands may use different base partitions
                    nc.vector.tensor_tensor(
                        yT[orows, h // 2, qs],
                        av_ps[h][orows], r2[srows], op=MUL)
                pend = []
                # post-norm work item: gives the PE queue something to chew
                # while the DVE norm muls drain (next pair's first AV
                # matmul WAR-waits on them through the 2-buf av pool)
                if wi[0] < nwork:
                    work[wi[0]]()
                    wi[0] += 1
                else:
                    # no interleavable work left (t=3): a few dummies keep
                    # the PE p-state alive through the norm chain
                    keepwarm(4)

            # ---- prologue: stream A for tile 0, undiluted ----
            for sub in a_subunits(0):
                sub()

            # ---- main: B(t) carrying A(t+1) and C(t-1) ----
            for t in range(T):
                work = []
                if t >= 1:
                    work += c_subunits(t - 1)
                if t <= 2:
                    work += a_subunits(t + 1)
                if t == 0:
                    nc.sync.dma_start(
                        out=wo_t[:],
                        in_=wo.rearrange("(c p) d -> p c d", p=128))
                wi = [0]
                for pair in range(2):
                    b_pair(t, pair, work, wi)
                while wi[0] < len(work):
                    work[wi[0]]()
                    wi[0] += 1

            # ---- tail: C(3) ----
            for i, sub in enumerate(c_subunits(3, tail=True)):
                sub()

    nc.compile()
    return nc


def _host_prep(x, token_positions, Wq, Wk, Wv, Wo):
    # d_k permutation folded into Wq/Wk.  Projection-output row n (0..255):
    # chunk oc = n//128 (all x1 lanes in chunk 0, x2 in chunk 1 for RoPE),
    # head h = (n%128)//32, freq j = n%32 -> orig row 64h + 2j + oc.
    # (The post-RoPE repack DMA then rearranges rows per-head-contiguous.)
    n = np.arange(GD)
    chunk = n // 128
    hh = (n % 128) // 32
    jj = n % 32
    perm = 64 * hh + 2 * jj + chunk

    pos = np.asarray(token_positions).astype(np.float64)
    inv_freq = THETA ** (-np.arange(0, DK, 2, dtype=np.float64) / DK)  # [32]
    ang = pos[:, None] * inv_freq[None, :]                             # [S, 32]
    cos = np.cos(ang).astype(np.float32)
    sin = np.sin(ang).astype(np.float32)
    # [128, S]: rows = freq j, replicated x4 on the host so the device
    # loads each table with a single DMA
    cc = np.ascontiguousarray(np.tile(cos.T, (4, 1)))
    ss = np.ascontiguousarray(np.tile(sin.T, (4, 1)))

    # causal mask tiles for diagonal-crossing chunks: msk[p, j, :, f] = 1.0
    # iff f >= p + 128*j (duplicated across the head-pair dim)
    pp_, ff_ = np.arange(128)[:, None], np.arange(512)[None, :]
    msk1 = np.stack([(ff_ >= pp_ + 128 * j) for j in range(4)], 0)  # [4,128,512]
    msk = np.repeat(msk1.transpose(1, 0, 2)[:, :, None, :], 2, axis=2)

    scale = 1.0 / np.sqrt(np.float32(DK))
    if USE_BF16:
        import ml_dtypes
        mmnp = ml_dtypes.bfloat16
    else:
        mmnp = np.float32
    in_maps = []
    for core in range(8):
        b, g = divmod(core, 4)
        gsl = slice(GD * g, GD * (g + 1))
        in_maps.append({
            "xT": np.ascontiguousarray(np.asarray(x[b], np.float32).T).astype(mmnp),
            "wq": np.ascontiguousarray(
                (np.asarray(Wq[gsl], np.float32) * scale)[perm].T.astype(mmnp)),
            "wk": np.ascontiguousarray(np.asarray(Wk[gsl], np.float32)[perm].T.astype(mmnp)),
            "wv": np.ascontiguousarray(np.asarray(Wv[gsl], np.float32).T.astype(mmnp)),
            "wo": np.ascontiguousarray(np.asarray(Wo[:, gsl], np.float32).T.astype(mmnp)),
            "cc": cc,
            "ss": ss,
            "msk": np.ascontiguousarray(msk).astype(mmnp),
        })
    return in_maps


def kernel(x, token_positions, Wq, Wk, Wv, Wo, _trace=False, _result=[None],
           _tmpdir=None):
    if "nc" not in _cache:
        _cache["nc"] = _build_kernel()
    nc = _cache["nc"]
    in_maps = _host_prep(x, token_positions, Wq, Wk, Wv, Wo)
    res = None
    for attempt in range(3):
        try:
            res = run_bass_kernel_spmd(
                nc, in_maps, core_ids=list(range(8)), trace=_trace,
                tmpdir=_tmpdir)
            break
        except Exception:
            # transient NRT_EXEC_UNIT_UNRECOVERABLE device hiccups resolve
            # on retry
            if attempt == 2:
                raise
    _result[0] = res
    outs = np.stack([r["out"] for r in res.results])  # [8, S, D] fp16
    full = outs.reshape(B, 4, S, D).sum(axis=1, dtype=np.float32)
    return full


# revision 28
# speedup vs baseline: 1.0104x; 1.0002x over previous
"""Causal multihead self-attention with RoPE on 8 TRN2 NeuronCores.

Problem: B=2, S=2048, D=1024, H=16 heads, d_k=64, causal, RoPE theta=10000.

Sharding (Megatron-style, per hint): core c = 4*b + g handles batch b and the
4 heads [4g, 4g+4): Wq/Wk/Wv column-parallel (sliced rows of W since torch
computes x @ W.T), Wo row-parallel; each core emits a partial [S, D] output
(fp16) and the host sums the 4 partials per batch in fp32.

Device kernel (per core); matmul operands in bf16 (fp32 PSUM accumulation,
fast weight loads), everything else fp32.  Three work streams:
  A) qT/kT = W' @ x.T in a d-on-partition layout; RoPE via 6 full-width DVE
     ops per (tensor, qtile) against host-built cos/sin tables; an 8-issue
     SBUF->SBUF DMA repack rearranges rows per-head-contiguous; v packed on
     ACT as per-head [v | ones] ([ones | v] for odd heads; the ones columns
     produce softmax sums for free in the attention*V matmul).
  B) Per (qtile, pair): scoresT [128 kpos, 2, 512 q] via PE row-group
     packing (2 heads concurrent), exp on ACT (|scores|<=~40, no max
     subtraction), DVE causal-mask multiply on diagonal tiles, AV
     accumulation with a 4-chunk software skew.  At pair end the
     remaining AV chunks drain per-head with each head's normalization
     (ACT ln/exp + one DVE mul into yT) emitted immediately after its
     drain, so head 0's norm hides under head 1's matmuls and the next
     pair's first AV (WAR on the 2-buf av pool) unblocks ~2us earlier.
  C) partial = yT.T @ Wo via PE, fp16 SBUF staging, one DMA per 128-row
     block alternating sync/gpsimd queues.

The defining constraint (from trace analysis): stream A is PE+DVE-heavy
with ACT idle, stream B is ACT-bound (exp ~71us) with PE ~50%.  Run
serially they floor at ~165us; this kernel INTERLEAVES them -- B(t) chunk
emissions carry A(t+1) and C(t-1) subunits distributed between them -- so
the ACT-bound stretch hides A's PE/DVE work.  Head-of-line stalls (the
failure mode of naive weaving, 212-218us in a previous attempt) are
avoided by (a) every interleaved instruction's data deps being satisfied
at least one full tile earlier, (b) disjoint PSUM budgets: scores 2x2
banks, AV 2 banks, A-projections/C-outputs share a 2-bank pool.

Other measured findings baked in:
  - Every dma_start costs ~600ns of sequencer time on the issuing engine's
    queue (the old ~95 sync-queue issues serialized ~60us); transfers fan
    out across all 16 DMA engines regardless, so issues are batched --
    except the startup-critical xt0/wq which are split per-chunk so the
    first projection matmuls start as soon as chunk 0 lands.
  - DVE tensor_tensor is ~0.67ns/elem regardless of dtype/space (no 2x/4x
    modes materialize on HW); vector.reciprocal is ~6.5 cycles/elem (far
    slower than the ACT ln/exp pair).
  - The BIR verifier requires SBUF DMA APs to keep the partition dim
    outermost; partition-block-strided out APs wedge the device
    (NRT_EXEC_UNIT_UNRECOVERABLE).
  - PE p-state drops to ~1.2GHz after idle gaps (~3us continuous execution
    to ramp); a dense interleaved stream is the fix, plus a short warmup
    during the initial DMA wait.
"""
import sys

sys.path.insert(0, "/opt/trn_rl_repo")

import numpy as np

import concourse.bacc as bacc
import concourse.hw_specs as hw_specs
import concourse.tile as tile
from concourse import mybir
from concourse.bass_utils import run_bass_kernel_spmd

# Keep Exp and Ln in one activation-table set: hide them from every other set
# so bacc's table-load pass picks natural_log_exp_and_others for both instead
# of thrashing between exp_and_others and natural_log (~2.7us per reload).
_orig_act_tables = hw_specs.get_activation_tables


def _patched_act_tables(arch):
    _E = mybir.ActivationFunctionType.Exp
    _L = mybir.ActivationFunctionType.Ln
    out = {}
    for name, fns in _orig_act_tables(arch).items():
        if name != "natural_log_exp_and_others":
            fns = fns - {_E, _L}
        out[name] = fns
    return out


bacc.get_activation_tables = _patched_act_tables

F32 = mybir.dt.float32
F32R = mybir.dt.float32r
F16 = mybir.dt.float16
BF16 = mybir.dt.bfloat16
USE_BF16 = True
MM_DT = BF16 if USE_BF16 else F32R
EXP = mybir.ActivationFunctionType.Exp
LN = mybir.ActivationFunctionType.Ln
MUL = mybir.AluOpType.mult
ADD = mybir.AluOpType.add
SUB = mybir.AluOpType.subtract

B, S, D = 2, 2048, 1024
H, DK = 16, 64          # global heads, head dim
HL = 4                  # heads per core
GD = HL * DK            # local width 256
T = S // 512            # 4 q-tiles of 512
C = S // 128            # 16 kpos chunks of 128
DCH = D // 128          # 8 contraction chunks
THETA = 10000.0

_cache = {}


def _build_kernel():
    nc = bacc.Bacc("TRN2", target_bir_lowering=False, debug=False, num_devices=8)

    xT = nc.declare_dram_parameter("xT", [D, S], MM_DT, isOutput=False)
    wq = nc.declare_dram_parameter("wq", [D, GD], MM_DT, isOutput=False)
    wk = nc.declare_dram_parameter("wk", [D, GD], MM_DT, isOutput=False)
    wv = nc.declare_dram_parameter("wv", [D, GD], MM_DT, isOutput=False)
    wo = nc.declare_dram_parameter("wo", [GD, D], MM_DT, isOutput=False)
    ccd = nc.declare_dram_parameter("cc", [128, S], F32, isOutput=False)
    ssd = nc.declare_dram_parameter("ss", [128, S], F32, isOutput=False)
    mskd = nc.declare_dram_parameter("msk", [128, 4, 2, 512], MM_DT,
                                     isOutput=False)
    out = nc.declare_dram_parameter("out", [S, D], F16, isOutput=True)

    with tile.TileContext(nc) as tc:
        with (
            tc.tile_pool(name="consts", bufs=1) as consts,
            tc.tile_pool(name="persist", bufs=1) as persist,
            tc.tile_pool(name="xtp", bufs=4) as xtp,
            tc.tile_pool(name="rtmp", bufs=12) as rtmp,
            tc.tile_pool(name="rop", bufs=4) as rop,
            tc.tile_pool(name="ep", bufs=10) as ep,
            tc.tile_pool(name="rp", bufs=10) as rp,
            tc.tile_pool(name="osb", bufs=3) as osb,
            # PSUM: scores 2x[128,2,512] (4 banks) + AV 2x[128,512]
            # (2 banks) + a 2-bank pool shared by A-projections, C-outputs
            # and warmup = exactly 8 banks
            tc.tile_pool(name="ap", bufs=2, space="PSUM") as ap,
            tc.tile_pool(name="scp", bufs=2, space="PSUM") as scp,
            tc.tile_pool(name="avp", bufs=2, space="PSUM") as avp,
        ):
            # ---- constants; xt0/wq split per-chunk so the first
            # projection matmuls start as soon as chunk 0 lands ----
            wq_t = consts.tile([128, DCH, GD], MM_DT, tag="wq")
            wk_t = consts.tile([128, DCH, GD], MM_DT, tag="wk")
            wv_t = consts.tile([128, DCH, GD], MM_DT, tag="wv")
            wo_t = consts.tile([128, 2, D], MM_DT, tag="wo")
            cc_t = consts.tile([128, S], F32, tag="cc")
            ss_t = consts.tile([128, S], F32, tag="ss")

            xts = {}
            for t in range(T):
                xts[t] = xtp.tile([128, DCH, 512], MM_DT, tag="xt",
                                  name=f"xt{t}")
            xT_v = xT.rearrange("(c p) s -> p c s", p=128)
            wq_v = wq.rearrange("(c p) g -> p c g", p=128)
            for dd in range(2):
                nc.sync.dma_start(out=xts[0][:, dd, :], in_=xT_v[:, dd, 0:512])
                nc.sync.dma_start(out=wq_t[:, dd, :], in_=wq_v[:, dd, :])
            nc.sync.dma_start(out=xts[0][:, 2:, :], in_=xT_v[:, 2:, 0:512])
            nc.sync.dma_start(out=wq_t[:, 2:, :], in_=wq_v[:, 2:, :])
            nc.gpsimd.dma_start(out=cc_t[:], in_=ccd[:])
            nc.gpsimd.dma_start(out=ss_t[:], in_=ssd[:])
            nc.sync.dma_start(
                out=wk_t[:], in_=wk.rearrange("(c p) g -> p c g", p=128))
            nc.sync.dma_start(
                out=wv_t[:], in_=wv.rearrange("(c p) g -> p c g", p=128))
            msk_t = consts.tile([128, 4, 2, 512], MM_DT, tag="msk")
            nc.sync.dma_start(out=msk_t[:], in_=mskd[:])
            # prefetch the remaining x tiles (transfers overlap compute)
            for t in range(1, T):
                nc.sync.dma_start(out=xts[t][:],
                                  in_=xT_v[:, :, 512 * t:512 * (t + 1)])

            # PE warmup during the initial DMA wait (the PE clock needs
            # ~3us of continuous execution to ramp to 2.4GHz)
            warm = consts.tile([128, 128], MM_DT, tag="warm")
            nc.vector.memset(warm[:], 0.03125)
            wps = ap.tile([128, 512], F32, tag="ap", name="warmps")
            NWARM = 28
            for i in range(NWARM):
                nc.tensor.matmul(wps[:, 0:128], lhsT=warm[:], rhs=warm[:],
                                 start=(i == 0), stop=(i == NWARM - 1))

            kw_n = [0]

            def keepwarm(n=2):
                kw_n[0] += 1
                wt = ap.tile([128, 512], F32, tag="ap",
                             name=f"kw{kw_n[0]}")
                for i in range(n):
                    nc.tensor.matmul(wt[:, 0:128], lhsT=warm[:],
                                     rhs=warm[:], start=True, stop=True)

            ones_f = consts.tile([128, 2, DK], F32, tag="onesf")
            nc.vector.memset(ones_f[:], 1.0)
            ones = consts.tile([128, 2, DK], MM_DT, tag="ones")
            nc.vector.tensor_copy(ones[:], ones_f[:])

            # persistent activations
            qT = persist.tile([128, 2, S], MM_DT, tag="qT")
            kT = persist.tile([128, 2, S], MM_DT, tag="kT")
            yT = persist.tile([128, 2, S], MM_DT, tag="yT")
            v_ext = persist.tile([128, C, HL, 2 * DK], MM_DT, tag="vext")

            # ones halves of v_ext: even heads [64:128], odd heads [0:64]
            for c in range(C):
                for par, sl in ((0, slice(DK, 2 * DK)), (1, slice(0, DK))):
                    nc.vector.tensor_copy(v_ext[:, c, par::2, sl], ones[:])

            # ---- stream A subunit emitters (projections + RoPE + v) ----
            def a_subunits(t):
                """10 closures: for q and k: proj-oc0+mul2, proj-oc1+mul3,
                rope-final+repack; then 4 v units."""
                qs = slice(512 * t, 512 * (t + 1))
                xt = xts[t]
                subs = []
                for wi_, (w_t, dst) in enumerate(((wq_t, qT), (wk_t, kT))):
                    st = {}
                    un = f"{t}_{wi_}"

                    def proj0(w_t=w_t, st=st, t=t, un=un):
                        ccs, sss = cc_t[:, qs], ss_t[:, qs]
                        st["t1"] = rtmp.tile([128, 512], F32, tag="rt",
                                             name=f"t1_{un}")
                        st["t2"] = rtmp.tile([128, 512], F32, tag="rt",
                                             name=f"t2_{un}")
                        st["t3"] = rtmp.tile([128, 512], F32, tag="rt",
                                             name=f"t3_{un}")
                        st["t4"] = rtmp.tile([128, 512], F32, tag="rt",
                                             name=f"t4_{un}")
                        st["ro"] = rop.tile([128, 2, 512], MM_DT, tag="ro",
                                            name=f"ro_{un}")
                        ps = ap.tile([128, 512], F32, tag="ap",
                                     name=f"ps0_{un}")
                        for d in range(DCH):
                            nc.tensor.matmul(
                                ps[:], lhsT=w_t[:, d, 0:128],
                                rhs=xt[:, d, :],
                                start=(d == 0), stop=(d == DCH - 1))
                        nc.vector.tensor_tensor(st["t1"][:], ps[:], ccs,
                                                op=MUL)
                        nc.vector.tensor_tensor(st["t3"][:], ps[:], sss,
                                                op=MUL)

                    def proj1(w_t=w_t, st=st, t=t, un=un):
                        ccs, sss = cc_t[:, qs], ss_t[:, qs]
                        ps = ap.tile([128, 512], F32, tag="ap",
                                     name=f"ps1_{un}")
                        for d in range(DCH):
                            nc.tensor.matmul(
                                ps[:], lhsT=w_t[:, d, 128:256],
                                rhs=xt[:, d, :],
                                start=(d == 0), stop=(d == DCH - 1))
                        nc.vector.tensor_tensor(st["t2"][:], ps[:], sss,
                                                op=MUL)
                        nc.vector.tensor_tensor(st["ro"][:, 0, :],
                                                st["t1"][:], st["t2"][:],
                                                op=SUB)
                        nc.vector.tensor_tensor(st["t4"][:], ps[:], ccs,
                                                op=MUL)

                    def ropef(dst=dst, st=st, t=t):
                        nc.vector.tensor_tensor(st["ro"][:, 1, :],
                                                st["t3"][:], st["t4"][:],
                                                op=ADD)
                        # repack to per-head-contiguous rows: dst rows
                        # 64*hp + 32*half + j, chunk oc, from ro rows
                        # 32*(2*oc+hp) + j, chunk half
                        dma_eng = nc.gpsimd if dst is qT else nc.sync
                        ro = st["ro"]
                        for half in range(2):
                            for oc in range(2):
                                for hp in range(2):
                                    sp = 32 * (2 * oc + hp)
                                    dp = 64 * hp + 32 * half
                                    dma_eng.dma_start(
                                        out=dst[dp:dp + 32, oc, qs],
                                        in_=ro[sp:sp + 32, half, :],
                                    )

                    subs += [proj0, proj1, ropef]

                for s4 in range(4):
                    def vunit(s4=s4, t=t):
                        s = 4 * t + s4
                        psv = ap.tile([128, 512], F32, tag="ap",
                                      name=f"psv_{t}_{s4}")
                        for d in range(DCH):
                            nc.tensor.matmul(
                                psv[:, :GD],
                                lhsT=xt[:, d, 128 * s4:128 * (s4 + 1)],
                                rhs=wv_t[:, d, :],
                                start=(d == 0), stop=(d == DCH - 1))
                        pv = psv[:, :GD].rearrange("p (h e) -> p h e", e=DK)
                        # v packing on ACT (slack there; DVE carries RoPE)
                        for par, sl in ((0, slice(0, DK)),
                                        (1, slice(DK, 2 * DK))):
                            nc.scalar.copy(
                                v_ext[:, s, par::2, sl], pv[:, par::2, :])
                    subs.append(vunit)
                return subs

            # ---- stream C subunit emitters (output projection) ----
            def c_subunits(t, tail=False):
                """8 closures, one per (row-block, half): po matmuls + fp16
                staging; the DMA (one per row-block) fires after half 1."""
                subs = []
                for s_ in range(4 * t, 4 * t + 4):
                    ob = osb.tile([128, 2, 512], F16, tag="ob",
                                  name=f"ob_{s_}")

                    def unit(s_=s_, ob=ob, n=0):
                        po = ap.tile([128, 512], F32, tag="ap",
                                     name=f"po_{s_}_{n}")
                        for ldc in range(2):
                            nc.tensor.matmul(
                                po[:],
                                lhsT=yT[:, ldc, 128 * s_:128 * (s_ + 1)],
                                rhs=wo_t[:, ldc, 512 * n:512 * (n + 1)],
                                start=(ldc == 0), stop=(ldc == 1))
                        if tail and n == 1:
                            # ACT is idle in the tail; splitting the copies
                            # shortens the serial epilogue chain
                            nc.scalar.copy(ob[:, n, :], po[:])
                        else:
                            nc.vector.tensor_copy(ob[:, n, :], po[:])
                        if n == 1:
                            nc.sync.dma_start(
                                out=out[128 * s_:128 * (s_ + 1), :],
                                in_=ob[:])

                    subs.append(lambda unit=unit: unit(n=0))
                    subs.append(lambda unit=unit: unit(n=1))
                return subs

            # ---- stream B emitter for one (t, pair) with interleaved
            # work items pulled between chunk emissions ----
            def b_pair(t, pair, work, wi):
                qs = slice(512 * t, 512 * (t + 1))
                heads = (2 * pair, 2 * pair + 1)
                av_ps = {h: avp.tile([128, 512], F32, tag="av",
                                     name=f"av_{t}_{h}")
                         for h in heads}
                nck = 4 * t + 4
                nwork = len(work)
                ntot = 8 * t + 8  # chunks in this tile (both pairs)
                pend = []
                for c in range(nck):
                    ks = slice(128 * c, 128 * (c + 1))
                    j = c - 4 * t
                    f0 = 128 * j if (0 < j < 4 and c > 0) else 0
                    sc = scp.tile([128, 2, 512], F32, tag="sc",
                                  name=f"sc_{t}_{pair}_{c}")
                    for hp in range(2):
                        rows = slice(64 * hp, 64 * hp + 64)
                        nc.tensor.matmul(
                            sc[:, hp, f0:],
                            lhsT=kT[rows, pair, ks],
                            rhs=qT[rows, pair, 512 * t + f0:512 * (t + 1)],
                            start=True, stop=True,
                            tile_position=(64 * hp, 0),
                        )
                    e = ep.tile([128, 2, 512], MM_DT, tag="e",
                                name=f"e_{t}_{pair}_{c}")
                    nc.scalar.activation(e[:, :, f0:], sc[:, :, f0:], EXP)
                    if c >= 4 * t:
                        nc.vector.tensor_tensor(
                            e[:, :, f0:], e[:, :, f0:],
                            msk_t[:, j, :, f0:], op=MUL)
                    pend.append((c, e, f0))
                    if len(pend) > 4:
                        pc, pe_, pf0 = pend.pop(0)
                        for hp, h in enumerate(heads):
                            nc.tensor.matmul(
                                av_ps[h][:, pf0:],
                                lhsT=v_ext[:, pc, h, :],
                                rhs=pe_[:, hp, pf0:],
                                start=(pc == 0), stop=False)
                    # pull interleaved A/C work: distribute the nwork
                    # items evenly across this tile's ntot chunks
                    done = pair * nck + c + 1
                    target = min(nwork, (done * nwork) // (ntot + 2))
                    while wi[0] < target:
                        work[wi[0]]()
                        wi[0] += 1
                last_pc = pend[-1][0]
                for hp, h in enumerate(heads):
                    for pc, pe_, pf0 in pend:
                        nc.tensor.matmul(
                            av_ps[h][:, pf0:],
                            lhsT=v_ext[:, pc, h, :],
                            rhs=pe_[:, hp, pf0:],
                            start=(pc == 0), stop=(pc == last_pc))
                    if h % 2 == 0:
                        srows, orows = slice(64, 128), slice(0, 64)
                    else:
                        srows, orows = slice(0, 64), slice(64, 128)
                    r1 = rp.tile([128, 512], F32, tag="rr",
                                 name=f"r1_{t}_{h}")
                    r2 = rp.tile([128, 512], F32, tag="rr",
                                 name=f"r2_{t}_{h}")
                    nc.scalar.activation(r1[srows], av_ps[h][srows], LN)
                    nc.scalar.activation(r2[srows], r1[srows], EXP,
                                         scale=-1.0)
                    # psum + sbuf operands may use different base partitions
                    nc.vector.tensor_tensor(
                        yT[orows, h // 2, qs],
                        av_ps[h][orows], r2[srows], op=MUL)
                pend = []
                # post-norm work item: gives the PE queue something to chew
                # while the DVE norm muls drain (next pair's first AV
                # matmul WAR-waits on them through the 2-buf av pool)
                if wi[0] < nwork:
                    work[wi[0]]()
                    wi[0] += 1
                else:
                    # no interleavable work left (t=3): a few dummies keep
                    # the PE p-state alive through the norm chain
                    keepwarm(4)

            # ---- prologue: stream A for tile 0, undiluted ----
            for sub in a_subunits(0):
                sub()

            # ---- main: B(t) carrying A(t+1) and C(t-1) ----
            for t in range(T):
                work = []
                if t >= 1:
                    work += c_subunits(t - 1)
                if t <= 2:
                    work += a_subunits(t + 1)
                if t == 0:
                    nc.sync.dma_start(
                        out=wo_t[:],
                        in_=wo.rearrange("(c p) d -> p c d", p=128))
                wi = [0]
                for pair in range(2):
                    b_pair(t, pair, work, wi)
                while wi[0] < len(work):
                    work[wi[0]]()
                    wi[0] += 1

            # ---- tail: C(3) ----
            for i, sub in enumerate(c_subunits(3, tail=True)):
                sub()

    nc.compile()
    return nc


def _host_prep(x, token_positions, Wq, Wk, Wv, Wo):
    # d_k permutation folded into Wq/Wk.  Projection-output row n (0..255):
    # chunk oc = n//128 (all x1 lanes in chunk 0, x2 in chunk 1 for RoPE),
    # head h = (n%128)//32, freq j = n%32 -> orig row 64h + 2j + oc.
    # (The post-RoPE repack DMA then rearranges rows per-head-contiguous.)
    n = np.arange(GD)
    chunk = n // 128
    hh = (n % 128) // 32
    jj = n % 32
    perm = 64 * hh + 2 * jj + chunk

    pos = np.asarray(token_positions).astype(np.float64)
    inv_freq = THETA ** (-np.arange(0, DK, 2, dtype=np.float64) / DK)  # [32]
    ang = pos[:, None] * inv_freq[None, :]                             # [S, 32]
    cos = np.cos(ang).astype(np.float32)
    sin = np.sin(ang).astype(np.float32)
    # [128, S]: rows = freq j, replicated x4 on the host so the device
    # loads each table with a single DMA
    cc = np.ascontiguousarray(np.tile(cos.T, (4, 1)))
    ss = np.ascontiguousarray(np.tile(sin.T, (4, 1)))

    # causal mask tiles for diagonal-crossing chunks: msk[p, j, :, f] = 1.0
    # iff f >= p + 128*j (duplicated across the head-pair dim)
    pp_, ff_ = np.arange(128)[:, None], np.arange(512)[None, :]
    msk1 = np.stack([(ff_ >= pp_ + 128 * j) for j in range(4)], 0)  # [4,128,512]
    msk = np.repeat(msk1.transpose(1, 0, 2)[:, :, None, :], 2, axis=2)

    scale = 1.0 / np.sqrt(np.float32(DK))
    if USE_BF16:
        import ml_dtypes
        mmnp = ml_dtypes.bfloat16
    else:
        mmnp = np.float32
    in_maps = []
    for core in range(8):
        b, g = divmod(core, 4)
        gsl = slice(GD * g, GD * (g + 1))
        in_maps.append({
            "xT": np.ascontiguousarray(np.asarray(x[b], np.float32).T).astype(mmnp),
            "wq": np.ascontiguousarray(
                (np.asarray(Wq[gsl], np.float32) * scale)[perm].T.astype(mmnp)),
            "wk": np.ascontiguousarray(np.asarray(Wk[gsl], np.float32)[perm].T.astype(mmnp)),
            "wv": np.ascontiguousarray(np.asarray(Wv[gsl], np.float32).T.astype(mmnp)),
            "wo": np.ascontiguousarray(np.asarray(Wo[:, gsl], np.float32).T.astype(mmnp)),
            "cc": cc,
            "ss": ss,
            "msk": np.ascontiguousarray(msk).astype(mmnp),
        })
    return in_maps


def kernel(x, token_positions, Wq, Wk, Wv, Wo, _trace=False, _result=[None],
           _tmpdir=None):
    if "nc" not in _cache:
        _cache["nc"] = _build_kernel()
    nc = _cache["nc"]
    in_maps = _host_prep(x, token_positions, Wq, Wk, Wv, Wo)
    res = None
    for attempt in range(3):
        try:
            res = run_bass_kernel_spmd(
                nc, in_maps, core_ids=list(range(8)), trace=_trace,
                tmpdir=_tmpdir)
            break
        except Exception:
            # transient NRT_EXEC_UNIT_UNRECOVERABLE device hiccups resolve
            # on retry
            if attempt == 2:
                raise
    _result[0] = res
    outs = np.stack([r["out"] for r in res.results])  # [8, S, D] fp16
    full = outs.reshape(B, 4, S, D).sum(axis=1, dtype=np.float32)
    return full
